# revision 7
# baseline (speedup 1.0000x reference)
"""DeltaLoss kernel for 8 TRN2 NeuronCores (Bass/Tile) — v2 quarter-matrix.

Problem: loss = 0.5*(CE_row + CE_col) over mma = 100 * unit(td) @ unit(im).T
where td/im are all ordered-pair deltas of txtf/imgf [96, 512] -> P = 9120.

v2 exploits the (i,j)<->(j,i) antisymmetry: with unordered pairs p',q'
(i<j, PQ = 4560) the ordered logits matrix is [[A,-A],[-A,A]] of the
quarter A[p',q'] = 100 * unit(td_p') . unit(im_q'). Row/col sums of
exp over the ordered matrix reduce to sums of e^x + e^-x = 2cosh(x)
over the quarter; we compute e^{|x|} instead (the dropped e^{-|x|}
term is bounded by PQ * 1 against sums >= e^{max|x|} ~ e^15 -> rel
err ~1e-5). Every lse and the diag is equal for a pair and its swap,
so the means over 9120 equal the means over 4560.

Per-core: p'-slice of 570 columns, loop over 36 q'-tiles of 128:
  PE  : logits psum tile = patq_tile^T @ HsT     (bf16, 570 free)
  DVE : in-place |x| on psum (tensor_scalar abs_max 0)
  ACT : e = Exp(|x| * invnim[q'] - 30) -> bf16, accum_out -> colsum
  PE  : rowsum psum += ones^T @ e  (accumulated across all 36 tiles)
Host combines: lse = 30 + log(sum), loss = mean(lse_row+lse_col)/2 - mean(diag).
"""

import os
import sys

import numpy as np

for _p in ("/opt/trn_rl_repo", "/root/.axon_site/_ro/trn_rl_repo"):
    if os.path.isdir(_p) and _p not in sys.path:
        sys.path.insert(0, _p)

N = 96
D = 512
PQ = N * (N - 1) // 2  # 4560 unordered pairs
NCORES = 8
PSL = PQ // NCORES  # 570
QT = (PQ + 127) // 128  # 36 q'-tiles; last tile has 80 rows
QTAIL = PQ - (QT - 1) * 128  # 80
SHIFT = 30.0
KCH = 128  # contraction chunk for the [96,96] gram matmuls (D=512 -> 4)

_CACHE = {}


def _pair_constants():
    import ml_dtypes

    ii, jj = np.triu_indices(N, k=1)
    patq = np.zeros((N, PQ), np.float32)
    patq[ii, np.arange(PQ)] = 1.0
    patq[jj, np.arange(PQ)] = -1.0
    return np.ascontiguousarray(patq.astype(ml_dtypes.bfloat16))


def _build(repeat=1):
    import concourse.bass as bass
    import concourse.tile as tile
    from concourse import bacc, mybir

    f32 = mybir.dt.float32
    f32r = mybir.dt.float32r
    bf16 = mybir.dt.bfloat16
    AF = mybir.ActivationFunctionType
    ALU = mybir.AluOpType

    nc = bacc.Bacc("TRN2", target_bir_lowering=False, debug=False,
                   num_devices=NCORES)

    d_txtT = nc.dram_tensor("txtT", [D, N], bf16, kind="ExternalInput").ap()
    d_imgT = nc.dram_tensor("imgT", [D, N], bf16, kind="ExternalInput").ap()
    d_patq = nc.dram_tensor("patq", [N, PQ], bf16, kind="ExternalInput").ap()
    d_psl = nc.dram_tensor("psl", [N, PSL], bf16, kind="ExternalInput").ap()
    d_diag = nc.dram_tensor("diag_o", [1, PSL], f32,
                            kind="ExternalOutput").ap()
    d_rowsum = nc.dram_tensor("rowsum_o", [1, PSL], f32,
                              kind="ExternalOutput").ap()
    d_colsum = nc.dram_tensor("colsum_o", [128, QT], f32,
                              kind="ExternalOutput").ap()

    chunks = [(0, 512), (512, PSL)]  # free-dim chunks of the 570 slice

    with tile.TileContext(nc) as tc:
        with tc.tile_pool(name="persist", bufs=1) as persist, \
             tc.tile_pool(name="pconst", bufs=1) as pconst:

            # ---- input DMAs ----
            img_sb = pconst.tile([KCH, D // KCH, N], bf16)
            nc.sync.dma_start(out=img_sb,
                              in_=d_imgT.rearrange("(a p) c -> p a c", p=KCH))
            txt_sb = pconst.tile([KCH, D // KCH, N], bf16)
            nc.sync.dma_start(out=txt_sb,
                              in_=d_txtT.rearrange("(a p) c -> p a c", p=KCH))
            psl_sb = pconst.tile([N, PSL], bf16)
            nc.sync.dma_start(out=psl_sb, in_=d_psl)
            patq_sb = persist.tile([N, PQ], bf16)
            for c0 in range(0, PQ, 1140):
                c1 = min(c0 + 1140, PQ)
                nc.sync.dma_start(out=patq_sb[:, c0:c1], in_=d_patq[:, c0:c1])

            # ---- constants ----
            ones96 = pconst.tile([N, 2], bf16)
            nc.vector.memset(ones96, 1.0)
            ones128 = pconst.tile([128, 1], bf16)
            nc.vector.memset(ones128, 1.0)
            negshift = pconst.tile([128, 1], f32)
            nc.vector.memset(negshift, -SHIFT)
            zeros128 = pconst.tile([128, 1], f32)
            nc.vector.memset(zeros128, 0.0)
            hundredsf = pconst.tile([1, N], f32)
            nc.vector.memset(hundredsf, 100.0)
            hundreds = pconst.tile([1, N], f32r)
            nc.vector.tensor_copy(hundreds, hundredsf)

            # ---- gram matrices (bf16 in, fp32 psum, bf16 out) ----
            with tc.tile_pool(name="gpsum", bufs=2, space="PSUM") as gpsum:
                def gram(lhs, rhs, tag):
                    ps = gpsum.tile([N, N], f32, tag="gram_ps")
                    for a in range(D // KCH):
                        nc.tensor.matmul(ps, lhsT=lhs[:, a, :],
                                         rhs=rhs[:, a, :],
                                         start=(a == 0),
                                         stop=(a == D // KCH - 1))
                    sb = pconst.tile([N, N], bf16, tag=tag)
                    nc.vector.tensor_copy(sb, ps)
                    return sb

                Ti_sb = gram(img_sb, img_sb, "Ti")
                Tt_sb = gram(txt_sb, txt_sb, "Tt")
                G_sb = gram(txt_sb, img_sb, "G")

            def rsqrt_newton(dst, v, pool, tag):
                # dst = 1/sqrt(v); ACT Sqrt polished by one Newton step.
                s = pool.tile(v.shape, f32, tag=f"{tag}_s")
                nc.scalar.activation(s, v, AF.Sqrt, bias=zeros128[:v.shape[0]])
                r = pool.tile(v.shape, f32, tag=f"{tag}_r")
                nc.vector.reciprocal(r, s)
                t1 = pool.tile(v.shape, f32, tag=f"{tag}_t1")
                nc.vector.tensor_mul(t1, r, r)
                nc.vector.tensor_mul(t1, t1, v)
                nc.vector.tensor_scalar(t1, t1, -0.5, 1.5, ALU.mult, ALU.add)
                nc.vector.tensor_mul(dst, r, t1)

            invnim_col = pconst.tile([128, QT], f32)
            HsT_sb = pconst.tile([N, PSL], bf16)
            colsum_sb = persist.tile([128, QT], f32)

            # ---- prep: nim2 (col layout) and HsT ----
            with tc.tile_pool(name="bpsum", bufs=2, space="PSUM") as bpsum, \
                 tc.tile_pool(name="wpsum", bufs=1, space="PSUM") as wpsum, \
                 tc.tile_pool(name="tpsum", bufs=1, space="PSUM") as tpsum, \
                 tc.tile_pool(name="npsum", bufs=1, space="PSUM") as npsum, \
                 tc.tile_pool(name="ppool", bufs=2) as ppool:
                nim2_ps = npsum.tile([128, 2 * QT], f32)

                # nim2[q'] = colsum of (Ti @ patq) * patq, col layout via
                # per-128-tile matmuls with ones.
                def nim2_chunks(ks):
                    for k in ks:
                        c0, c1 = 512 * k, min(512 * k + 512, PQ)
                        psc = bpsum.tile([N, 512], f32, tag="bchunk")
                        nc.tensor.matmul(psc[:, :c1 - c0], lhsT=Ti_sb,
                                         rhs=patq_sb[:, c0:c1], start=True,
                                         stop=True)
                        prod = ppool.tile([N, 512], bf16, tag="prod")
                        nc.vector.tensor_mul(prod[:, :c1 - c0],
                                             psc[:, :c1 - c0],
                                             patq_sb[:, c0:c1])
                        for t in range(c0 // 128, (c1 + 127) // 128):
                            h = min(128, c1 - t * 128)
                            o = t * 128 - c0
                            nc.tensor.matmul(nim2_ps[:h, 2 * t:2 * t + 2],
                                             lhsT=prod[:, o:o + h],
                                             rhs=ones96, start=True, stop=True)

                nim2_chunks(range(5))

                # HsT chain (interleaves on otherwise-idle ACT/PE)
                psB = wpsum.tile([N, PSL], f32, tag="wide")
                for c0, c1 in chunks:
                    nc.tensor.matmul(psB[:, c0:c1], lhsT=Tt_sb,
                                     rhs=psl_sb[:, c0:c1], start=True,
                                     stop=True)
                prodt = ppool.tile([N, PSL], bf16, tag="prodt")
                nc.vector.tensor_mul(prodt, psB, psl_sb)
                ntd2_ps = tpsum.tile([1, PSL], f32, tag="thin")
                for c0, c1 in chunks:
                    nc.tensor.matmul(ntd2_ps[:, c0:c1], lhsT=ones96[:, 0:1],
                                     rhs=prodt[:, c0:c1], start=True,
                                     stop=True)
                ntd2f = pconst.tile([1, PSL], f32)
                nc.vector.tensor_copy(ntd2f, ntd2_ps)
                # inv_ntd = 1/sqrt(ntd2): ACT sqrt + DVE reciprocal (no
                # Newton; table error ~0.2% -> ~1e-4 on the loss)
                ntds = pconst.tile([1, PSL], f32)
                nc.scalar.activation(ntds, ntd2f, AF.Sqrt, bias=zeros128[0:1])
                inv_ntdf = pconst.tile([1, PSL], f32)
                nc.vector.reciprocal(inv_ntdf, ntds)
                inv_ntd = pconst.tile([1, PSL], f32r)
                nc.vector.tensor_copy(inv_ntd, inv_ntdf)

                bc_ps = wpsum.tile([N, PSL], f32, tag="wide")
                for c0, c1 in chunks:
                    nc.tensor.matmul(bc_ps[:, c0:c1], lhsT=hundreds,
                                     rhs=inv_ntd[0:1, c0:c1],
                                     start=True, stop=True)
                bc_sb = pconst.tile([N, PSL], f32)
                nc.scalar.copy(bc_sb, bc_ps)

                psH = wpsum.tile([N, PSL], f32, tag="wide")
                for c0, c1 in chunks:
                    nc.tensor.matmul(psH[:, c0:c1], lhsT=G_sb,
                                     rhs=psl_sb[:, c0:c1], start=True,
                                     stop=True)
                nc.vector.tensor_mul(HsT_sb, psH, bc_sb)

                nim2_chunks(range(5, 9))

                # invnim_col = 1/sqrt(nim2) with Newton polish ([128, QT])
                v_col = pconst.tile([128, QT], f32)
                nc.vector.memset(v_col, 1.0)
                nc.vector.tensor_copy(
                    v_col[:, :QT - 1],
                    nim2_ps.rearrange("p (t two) -> p t two",
                                      two=2)[:, :QT - 1, 0])
                nc.vector.tensor_copy(
                    v_col[:QTAIL, QT - 1:QT],
                    nim2_ps[:QTAIL, 2 * (QT - 1):2 * (QT - 1) + 1])
                rsqrt_newton(invnim_col, v_col, pconst, "nimcol")

            # ---- main loop over 36 q'-tiles ----
            with tc.tile_pool(name="mpsum", bufs=2, space="PSUM") as mpsum, \
                 tc.tile_pool(name="rpsum", bufs=1, space="PSUM") as rpsum, \
                 tc.tile_pool(name="epool", bufs=3) as epool:
                rs_ps = [rpsum.tile([1, c1 - c0], f32, tag=f"rs{c0}",
                                    name=f"rs_ps{c0}")
                         for c0, c1 in chunks]
                import contextlib
                loop_cm = (tc.For_i(0, repeat, 1) if repeat != 1
                           else contextlib.nullcontext())
                with loop_cm:
                    for t in range(QT):
                        h = 128 if t < QT - 1 else QTAIL
                        ps = mpsum.tile([128, PSL], f32, tag="logits")
                        for c0, c1 in chunks:
                            nc.tensor.matmul(ps[:h, c0:c1],
                                             lhsT=patq_sb[:,
                                                          t * 128:t * 128 + h],
                                             rhs=HsT_sb[:, c0:c1],
                                             start=True, stop=True)
                        psi = ps[:h].bitcast(mybir.dt.int32)
                        nc.vector.tensor_scalar(psi, psi, 0x7FFFFFFF, None,
                                                ALU.bitwise_and)
                        e = epool.tile([128, PSL], bf16, tag="exp")
                        nc.scalar.activation(e[:h], ps[:h], AF.Exp,
                                             bias=negshift[:h],
                                             scale=invnim_col[:h, t:t + 1],
                                             accum_out=colsum_sb[:h, t:t + 1])
                        for ci, (c0, c1) in enumerate(chunks):
                            nc.tensor.matmul(rs_ps[ci], lhsT=ones128[:h],
                                             rhs=e[:h, c0:c1],
                                             start=(t == 0),
                                             stop=(t == QT - 1))

                # drain rowsum + colsum
                rowsum_sb = pconst.tile([1, PSL], f32)
                for ci, (c0, c1) in enumerate(chunks):
                    nc.scalar.copy(rowsum_sb[:, c0:c1], rs_ps[ci])
                nc.sync.dma_start(out=d_rowsum, in_=rowsum_sb)
                nc.sync.dma_start(out=d_colsum, in_=colsum_sb)

            # ---- diag (post-loop; engines drain after main loop) ----
            with tc.tile_pool(name="dpsum", bufs=2, space="PSUM") as dpsum, \
                 tc.tile_pool(name="dpool", bufs=1) as dpool:
                # inv_nim for the slice (free layout)
                psBi = dpsum.tile([N, PSL], f32, tag="wide")
                for c0, c1 in chunks:
                    nc.tensor.matmul(psBi[:, c0:c1], lhsT=Ti_sb,
                                     rhs=psl_sb[:, c0:c1], start=True,
                                     stop=True)
                prodi = dpool.tile([N, PSL], bf16, tag="prodi")
                nc.vector.tensor_mul(prodi, psBi, psl_sb)
                nims_ps = dpsum.tile([1, PSL], f32, tag="thin")
                for c0, c1 in chunks:
                    nc.tensor.matmul(nims_ps[:, c0:c1], lhsT=ones96[:, 0:1],
                                     rhs=prodi[:, c0:c1], start=True,
                                     stop=True)
                nimsf = dpool.tile([1, PSL], f32)
                nc.vector.tensor_copy(nimsf, nims_ps)
                nims = dpool.tile([1, PSL], f32)
                nc.scalar.activation(nims, nimsf, AF.Sqrt, bias=zeros128[0:1])
                inv_nim_sl = dpool.tile([1, PSL], f32)
                nc.vector.reciprocal(inv_nim_sl, nims)

                # diag = (psl . HsT colsums) * inv_nim  (100/ntd already in HsT)
                prodd = dpool.tile([N, PSL], bf16, tag="prodd")
                nc.vector.tensor_mul(prodd, psl_sb, HsT_sb)
                diag_ps = dpsum.tile([1, PSL], f32, tag="thin")
                for c0, c1 in chunks:
                    nc.tensor.matmul(diag_ps[:, c0:c1], lhsT=ones96[:, 0:1],
                                     rhs=prodd[:, c0:c1], start=True,
                                     stop=True)
                diag_sb = dpool.tile([1, PSL], f32)
                nc.vector.tensor_mul(diag_sb, diag_ps, inv_nim_sl)
                nc.sync.dma_start(out=d_diag, in_=diag_sb)

    nc.compile()
    return nc


def _get_nc():
    if "nc" not in _CACHE:
        _CACHE["nc"] = _build()
        _CACHE["patq"] = _pair_constants()
    return _CACHE["nc"], _CACHE["patq"]


def make_in_maps(txtf, imgf, patq):
    import ml_dtypes

    bf = ml_dtypes.bfloat16
    txtT = np.ascontiguousarray(np.asarray(txtf, np.float32).T.astype(bf))
    imgT = np.ascontiguousarray(np.asarray(imgf, np.float32).T.astype(bf))
    in_maps = []
    for c in range(NCORES):
        sl = patq[:, c * PSL:(c + 1) * PSL]
        in_maps.append({
            "txtT": txtT,
            "imgT": imgT,
            "patq": patq,
            "psl": np.ascontiguousarray(sl),
        })
    return in_maps


def kernel(txtf: np.ndarray, imgf: np.ndarray) -> np.ndarray:
    from concourse import bass_utils

    nc, patq = _get_nc()
    in_maps = make_in_maps(txtf, imgf, patq)

    res = bass_utils.run_bass_kernel_spmd(
        nc, in_maps, core_ids=list(range(NCORES)))
    outs = res.results

    diag = np.concatenate([outs[c]["diag_o"][0] for c in range(NCORES)])
    rowsum = np.concatenate([outs[c]["rowsum_o"][0] for c in range(NCORES)])
    colsum_col = np.zeros((128, QT), np.float64)
    for c in range(NCORES):
        colsum_col += outs[c]["colsum_o"].astype(np.float64)
    colsum = np.zeros(PQ, np.float64)
    colsum[:(QT - 1) * 128] = colsum_col[:, :QT - 1].T.reshape(-1)
    colsum[(QT - 1) * 128:] = colsum_col[:QTAIL, QT - 1]

    lse_row = SHIFT + np.log(rowsum.astype(np.float64))
    lse_col = SHIFT + np.log(colsum)
    loss1 = np.mean(lse_row - diag)
    loss2 = np.mean(lse_col - diag)
    return np.float32(0.5 * (loss1 + loss2))
